# revision 68
# baseline (speedup 1.0000x reference)
"""Trainium2 Bass kernel for nn_DCConv3dKernelPolynomials.

out[o,i,x,n] = sum_b basis_b(position[x,n]) * coeffs[o,i,b]

Strategy: shard the 110592 grid points across the 8 NeuronCores (13824 each),
replicate the folded coefficient matrix.  The host re-encodes each point as
[sin t, cos t, sin p, cos p, sin2p/2, cos2p/2, sin3p/4, cos3p/4,
 r/2, r/3, r/4, r] (a coordinate re-parametrization, like the baseline's
host-folded normalizations) so the device needs only the Exp activation
table (one table load, hidden under the input DMAs).  Per core:
 - one fat Exp activation per slice produces all four radial exponentials,
 - a DMA'd root table + one fat DVE subtract yields all (r - root_k)
   factors; radial ladders are then plain TT multiplies (Pool),
 - broadcast-AP fat tensor ops assemble the angular factors (depth <= 3
   from the pos columns) and write the 30 basis values point-major into a
   bf16 tile; constants are folded into the coefficients host-side,
 - PE transposes (bf16, 1 cyc/row, with low-priority filler transposes
   keeping the p-state ramp warm) + 2x-mode DVE copies build psi in
   (30 x points) layout,
 - bf16 band matmuls (4 K=30 row bands via tile_position, each output
   PSUM-bank-aligned -- the hardware rejects sub-bank matmul outputs)
   against the replicated bf16 coefficients accumulate in PSUM f32,
 - ACT/DVE evacuations cast PSUM to bf16 staging (GPSIMD cannot touch
   PSUM), one DMA per 512-column chunk writes out; the pipeline is
   software-scheduled via emission priorities + tile_wait_until hints.
"""
import math

import numpy as np

OUTC, INC = 16, 16
OUTN, CONV_N = 4096, 27
NB = 30
NCORES = 8
PTS = OUTN * CONV_N            # 110592
CPTS = PTS // NCORES           # 13824 per core
NGRP = 4                       # point groups j (matmul row bands)
NROUND = 27                    # transpose rounds per group
F = NROUND * NGRP              # 108 g-columns, g = 4*c + j
GPTS = CPTS // NGRP            # 3456 psiT columns per core

# pos column indices (host sends theta/phi harmonics + scaled radii)
S_TH, C_TH, S_PH, C_PH, S2P, C2P, S3P, C3P, RH, RT3, RQ, RR = range(12)
NPC = 12

# g-col ranges: derived slices and matmul/DMA chunks (pos pieces are
# (0,8) (8,16) (16,32) (32,108); slices never cross pieces)
SLICES = [(0, 8), (8, 16), (16, 32), (32, 64), (64, 108)]
CHUNKS = [(0, 8), (8, 16), (16, 32), (32, 48), (48, 64), (64, 80),
          (80, 96), (96, 108)]


# ----------------------------------------------------------------- constants
def _qnums():
    lst = []
    for n in range(1, 5):
        for l in range(0, min(n, 4)):
            for m in range(-l, l + 1):
                if abs(m) <= 3:
                    lst.append((n, l, m))
    return lst


QNUMS = _qnums()


def _laguerre_coeffs(k, alpha):
    return [((-1.0) ** i) * math.comb(k + alpha, k - i) / math.factorial(i)
            for i in range(k + 1)]


def _radial_info(n, l):
    k = n - l - 1
    lag = _laguerre_coeffs(k, 2 * l + 1)
    cr = [lag[i] * (2.0 / n) ** i for i in range(k + 1)]
    norm_r = math.sqrt((2.0 / n) ** 3 * math.factorial(n - l - 1)
                       / (2.0 * n * math.factorial(n + l)))
    lead = cr[-1]
    K_rad = norm_r * (2.0 / n) ** l * lead
    roots = [] if k == 0 else sorted(float(x) for x in
                                     np.real(np.roots(np.array(cr[::-1]))))
    return roots, K_rad


_K_ANG = {(0, 0): 1.0, (1, 0): 1.0, (1, 1): -1.0,
          (2, 0): 1.5, (2, 1): -3.0, (2, 2): 3.0,
          (3, 0): 2.5, (3, 1): -7.5, (3, 2): 15.0, (3, 3): -15.0}
_TRIGFOLD = {0: 1.0, 1: 1.0, -1: 1.0, 2: 2.0, -2: 2.0, 3: 4.0, -3: 4.0}

ROOTS20 = _radial_info(2, 0)[0]
ROOTS30 = _radial_info(3, 0)[0]
ROOTS31 = _radial_info(3, 1)[0]
ROOTS40 = _radial_info(4, 0)[0]
ROOTS41 = _radial_info(4, 1)[0]
ROOTS42 = _radial_info(4, 2)[0]
# root-shift table: RS[k] = r - ROOTLIST[k] (one fat subtract per slice)
ROOTLIST = [ROOTS20[0], ROOTS30[0], ROOTS30[1], ROOTS31[0], ROOTS40[0],
            ROOTS40[1], ROOTS40[2], ROOTS41[0], ROOTS41[1], ROOTS42[0]]
NRT = len(ROOTLIST)


def _fold_constants():
    K = np.zeros(NB)
    for b, (n, l, m) in enumerate(QNUMS):
        am = abs(m)
        _, K_rad = _radial_info(n, l)
        klm = math.sqrt((2.0 * l + 1.0) / (4.0 * math.pi)
                        * math.factorial(l - am) / math.factorial(l + am))
        K[b] = (K_rad * klm * (math.sqrt(2.0) if m != 0 else 1.0)
                * _K_ANG[(l, am)] * _TRIGFOLD[m])
    return K


# ------------------------------------------------------------- device program
_PROGRAM_CACHE = {}
PSI_BF16 = True       # bf16 psi transpose path (f32r weights x bf16 rhs)
PHASE = [""]          # current emission phase label (for trace attribution)
PHASELOG = []         # (label, bass_priority at emission start)
_TC = [None]


def _mark(label):
    PHASE[0] = label
    if _TC[0] is not None:
        PHASELOG.append((label, _TC[0].cur_priority))


def _build_program():
    import concourse.bacc as bacc
    import concourse.tile as tile
    from concourse import mybir

    f32 = mybir.dt.float32
    bf16 = mybir.dt.bfloat16

    nc = bacc.Bacc("TRN2", debug=False, num_devices=NCORES)

    posA0_d = nc.dram_tensor("posA0", [128, NPC, 8], f32,
                             kind="ExternalInput")
    posA1_d = nc.dram_tensor("posA1", [128, NPC, 8], f32,
                             kind="ExternalInput")
    posB1_d = nc.dram_tensor("posB1", [128, NPC, 16], f32,
                             kind="ExternalInput")
    posB2_d = nc.dram_tensor("posB2", [128, NPC, 76], f32,
                             kind="ExternalInput")
    wts_d = nc.dram_tensor("wts", [128, 256],
                           bf16 if PSI_BF16 else f32, kind="ExternalInput")
    identdt = bf16 if PSI_BF16 else f32
    ident_d = nc.dram_tensor("ident", [128, 128], identdt,
                             kind="ExternalInput")
    roots_d = nc.dram_tensor("rootsT", [128, NRT], f32, kind="ExternalInput")
    out_d = nc.dram_tensor("out", [128, 2, NGRP, GPTS], bf16,
                           kind="ExternalOutput")

    with tile.TileContext(nc) as tc:
        _TC[0] = tc
        _kernel_body(tc, nc, out_d.ap(), posA0_d.ap(), posA1_d.ap(),
                     posB1_d.ap(), posB2_d.ap(), wts_d.ap(), ident_d.ap(),
                     roots_d.ap())
    nc.compile()
    return nc


def _kernel_body(tc, nc, out_ap, posA0_ap, posA1_ap, posB1_ap, posB2_ap,
                 wts_ap, ident_ap, roots_ap):
    from contextlib import ExitStack
    from concourse import mybir
    Alu = mybir.AluOpType
    AF = mybir.ActivationFunctionType
    f32 = mybir.dt.float32
    bf16 = mybir.dt.bfloat16

    ctx = ExitStack()
    with ctx:
        const = ctx.enter_context(tc.tile_pool(name="const", bufs=1))
        feat = ctx.enter_context(tc.tile_pool(name="feat", bufs=1))
        pT = ctx.enter_context(tc.tile_pool(name="pT", bufs=1, space="PSUM"))
        pM = ctx.enter_context(tc.tile_pool(name="pM", bufs=3, space="PSUM"))
        pW = ctx.enter_context(tc.tile_pool(name="pW", bufs=1, space="PSUM"))
        stg = ctx.enter_context(tc.tile_pool(name="stg", bufs=3))

        act = nc.scalar.activation
        scopy = nc.scalar.copy
        vtt = nc.vector.tensor_tensor
        vts = nc.vector.tensor_scalar
        vstt = nc.vector.scalar_tensor_tensor
        vcopy = nc.vector.tensor_copy
        gtt = nc.gpsimd.tensor_tensor
        gstt = nc.gpsimd.scalar_tensor_tensor
        gcopy = nc.gpsimd.tensor_copy

        # ------- input DMAs spread across SEQ queues (dispatch is ~1.3us) --
        posA0 = feat.tile([128, NPC, 8], f32)
        posA1 = feat.tile([128, NPC, 8], f32)
        posB1 = feat.tile([128, NPC, 16], f32)
        posB2 = feat.tile([128, NPC, 76], f32)
        psidt = bf16 if PSI_BF16 else f32
        wts = const.tile([128, 256], bf16 if PSI_BF16 else f32)
        ident = const.tile([128, 128], psidt)
        nc.sync.dma_start(posA0[:], posA0_ap)
        nc.sync.dma_start(posB1[:], posB1_ap)
        nc.sync.dma_start(posB2[:], posB2_ap)
        nc.scalar.dma_start(posA1[:], posA1_ap)
        RT = const.tile([128, NRT], f32)
        nc.scalar.dma_start(RT[:], roots_ap)
        nc.gpsimd.dma_start(wts[:], wts_ap)
        nc.gpsimd.dma_start(ident[:], ident_ap)
        if PSI_BF16:
            wtsr = wts
        else:
            wtsr = const.tile([128, 256], mybir.dt.float32r)

        def P(c, a, b):
            """pos column c over g-range [a,b) (never crosses pieces)."""
            if b <= 8:
                return posA0[:, c, a:b]
            if b <= 16:
                return posA1[:, c, a - 8:b - 8]
            if b <= 32:
                return posB1[:, c, a - 16:b - 16]
            return posB2[:, c, a - 32:b - 32]

        def Pblk(c0, c1, a, b):
            if b <= 8:
                return posA0[:, c0:c1, a:b]
            if b <= 16:
                return posA1[:, c0:c1, a - 8:b - 8]
            if b <= 32:
                return posB1[:, c0:c1, a - 16:b - 16]
            return posB2[:, c0:c1, a - 32:b - 32]

        # ---------------- full-F working tiles ----------------------------
        E = feat.tile([128, 4, F], f32)     # [E2, E3, E4, P0=e^-r]
        u2 = feat.tile([128, F], f32)
        stsq = feat.tile([128, F], f32)
        p33 = feat.tile([128, F], f32)
        q6 = feat.tile([128, F], f32)
        q7 = feat.tile([128, F], f32)
        A1 = feat.tile([128, 3, F], f32)    # [st*sp, ct, st*cp]
        A2 = feat.tile([128, 5, F], f32)
        A3 = feat.tile([128, 7, F], f32)
        RS = feat.tile([128, NRT, F], f32)  # r - root_k
        Er = feat.tile([128, 3, F], f32)    # [E2r, E3r, E4r]
        T4 = feat.tile([128, 2, F], f32)    # [R32, E4r2]
        R43 = feat.tile([128, F], f32)
        R31 = feat.tile([128, F], f32)
        R41a = feat.tile([128, F], f32)
        R41 = feat.tile([128, F], f32)
        R42 = feat.tile([128, F], f32)
        R30a = feat.tile([128, F], f32)
        R40a = feat.tile([128, F], f32)
        R40b = feat.tile([128, F], f32)

        PM = feat.tile([128, F, 32], psidt)
        psiT = feat.tile([128, GPTS],
                         bf16 if PSI_BF16 else mybir.dt.float32r)
        nc.vector.memset(PM[:, :, 30:32], 0.0)

        def bc(ap2d, k):
            """[128, n] -> [128, n, k] stride-0 broadcast."""
            return ap2d.unsqueeze(2).broadcast_to([128, ap2d.ap[-1][1], k])

        # ---------------- per-slice seed/derived ops ----------------------
        def emit_seeds(si):
            a, b = SLICES[si]
            _mark(f"seed{si}")
            act(E[:, :, a:b], Pblk(8, 12, a, b), AF.Exp, scale=-1.0)
            # squares: DVE on the first slice (latency), ACT afterwards
            if si < 1:
                vtt(u2[:, a:b], P(C_TH, a, b), P(C_TH, a, b), Alu.mult)
                vtt(stsq[:, a:b], P(S_TH, a, b), P(S_TH, a, b), Alu.mult)
            else:
                act(u2[:, a:b], P(C_TH, a, b), AF.Square)
                act(stsq[:, a:b], P(S_TH, a, b), AF.Square)
            # radial ladder: fat root-shift on DVE, multiplies on Pool (TT
            # only -- GPSIMD has no TensorScalar/STT and cannot touch PSUM)
            n = b - a
            rb = lambda k: bc(P(RR, a, b), k)
            vtt(RS[:, :, a:b],
                P(RR, a, b).unsqueeze(1).broadcast_to([128, NRT, n]),
                RT[:].unsqueeze(2).broadcast_to([128, NRT, n]),
                Alu.subtract)
            gtt(Er[:, :, a:b].transpose([0, 2, 1]), rb(3),
                E[:, 0:3, a:b].transpose([0, 2, 1]), Alu.mult)
            gtt(T4[:, :, a:b].transpose([0, 2, 1]), rb(2),
                Er[:, 1:3, a:b].transpose([0, 2, 1]), Alu.mult)
            gtt(R43[:, a:b], P(RR, a, b), T4[:, 1, a:b], Alu.mult)
            gtt(R31[:, a:b], RS[:, 3, a:b], Er[:, 1, a:b], Alu.mult)
            gtt(R41a[:, a:b], RS[:, 7, a:b], Er[:, 2, a:b], Alu.mult)
            gtt(R41[:, a:b], RS[:, 8, a:b], R41a[:, a:b], Alu.mult)
            gtt(R42[:, a:b], RS[:, 9, a:b], T4[:, 1, a:b], Alu.mult)
            gtt(R30a[:, a:b], RS[:, 1, a:b], E[:, 1, a:b], Alu.mult)
            gtt(R40a[:, a:b], RS[:, 4, a:b], E[:, 2, a:b], Alu.mult)
            gtt(R40b[:, a:b], RS[:, 5, a:b], R40a[:, a:b], Alu.mult)
            # angular ladder, depth <= 3 from pos columns:
            #   L2: p33, q6, q7, A1, p20->A2[2], p30->A3[3]
            #   L3: A2/A3 fat rows from pos harmonics
            att = vtt if si < 2 else gtt
            att(p33[:, a:b], P(S_TH, a, b), stsq[:, a:b], Alu.mult)
            att(q6[:, a:b], P(C_TH, a, b), stsq[:, a:b], Alu.mult)
            vstt(q7[:, a:b], u2[:, a:b], 0.2, P(S_TH, a, b),
                 Alu.subtract, Alu.mult)
            vcopy(A1[:, 1, a:b], P(C_TH, a, b))
            att(A1[:, 0:3:2, a:b].transpose([0, 2, 1]),
                bc(P(S_TH, a, b), 2),
                Pblk(S_PH, C_PH + 1, a, b).transpose([0, 2, 1]), Alu.mult)
            att(A2[:, 1:4:2, a:b].transpose([0, 2, 1]),
                bc(P(C_TH, a, b), 2),
                A1[:, 0:3:2, a:b].transpose([0, 2, 1]), Alu.mult)
            vts(A2[:, 2, a:b], u2[:, a:b], 1.0, -1.0 / 3.0, Alu.mult, Alu.add)
            att(A2[:, 0:5:4, a:b].transpose([0, 2, 1]),
                bc(stsq[:, a:b], 2),
                Pblk(S2P, C2P + 1, a, b).transpose([0, 2, 1]), Alu.mult)
            att(A3[:, 1:6:4, a:b].transpose([0, 2, 1]),
                bc(q6[:, a:b], 2),
                Pblk(S2P, C2P + 1, a, b).transpose([0, 2, 1]), Alu.mult)
            att(A3[:, 2:5:2, a:b].transpose([0, 2, 1]),
                bc(q7[:, a:b], 2),
                Pblk(S_PH, C_PH + 1, a, b).transpose([0, 2, 1]), Alu.mult)
            vstt(A3[:, 3, a:b], u2[:, a:b], 0.6, P(C_TH, a, b),
                 Alu.subtract, Alu.mult)
            att(A3[:, 0:7:6, a:b].transpose([0, 2, 1]),
                bc(p33[:, a:b], 2),
                Pblk(S3P, C3P + 1, a, b).transpose([0, 2, 1]), Alu.mult)

        # ---------------- per-chunk products into PM ----------------------
        def emit_products(ci):
            a, b = CHUNKS[ci]
            _mark(f"prod{ci}")

            def fat(engine_tt, b0, nsl, rad1d, angT):
                engine_tt(PM[:, a:b, b0:b0 + nsl], bc(rad1d, nsl), angT,
                          Alu.mult)

            A1T = A1[:, :, a:b].transpose([0, 2, 1])
            A2T = A2[:, :, a:b].transpose([0, 2, 1])
            A3T = A3[:, :, a:b].transpose([0, 2, 1])
            fat(vtt, 2, 3, Er[:, 0, a:b], A1T)          # E2r x A1
            fat(gtt, 6, 3, R31[:, a:b], A1T)            # R31 x A1
            fat(vtt, 9, 5, T4[:, 0, a:b], A2T)          # R32 x A2
            fat(gtt, 15, 3, R41[:, a:b], A1T)           # R41 x A1
            fat(vtt, 18, 5, R42[:, a:b], A2T)           # R42 x A2
            fat(vtt, 23, 7, R43[:, a:b], A3T)           # R43 x A3
            # l=0 finals + P0
            gtt(PM[:, a:b, 1], RS[:, 0, a:b], E[:, 0, a:b], Alu.mult)
            gtt(PM[:, a:b, 5], RS[:, 2, a:b], R30a[:, a:b], Alu.mult)
            gtt(PM[:, a:b, 14], RS[:, 6, a:b], R40b[:, a:b], Alu.mult)
            vcopy(PM[:, a:b, 0], E[:, 3, a:b])

        # ---------------- transpose + psiT copy per chunk -----------------
        def emit_tp(ci, cp="d"):
            a, b = CHUNKS[ci]
            _mark(f"tp{ci}")
            nr = (b - a) // 4
            tp = pT.tile([128, nr, 128], psidt, tag="tp")
            for i, c in enumerate(range(a // 4, b // 4)):
                nc.tensor.transpose(tp[:, i, :], PM[:, 4 * c:4 * c + 4, :],
                                    ident[:])
            dst = psiT[:, 32 * a:32 * b]
            srcap = tp[:].rearrange("p a b -> p (a b)")
            (vcopy if cp == "d" else scopy)(dst, srcap)

        # ---------------- matmul + evac + DMA per chunk -------------------
        outv = out_ap
        ENG = {"a": scopy, "d": vcopy, "p": gcopy}

        def emit_mm(ci, ev, split_h_dma=False):
            a, b = CHUNKS[ci]
            _mark(f"mm{ci}")
            C = 32 * (b - a)
            col0 = 32 * a
            so = stg.tile([128, 2, NGRP, C], bf16, tag="so")
            ev = list(ev)

            def mm(ps_ap, j, h):
                lhsT = wtsr[32 * j:32 * j + NB, 128 * h:128 * (h + 1)]
                rhs = psiT[32 * j:32 * j + NB, col0:col0 + C]
                nc.tensor.matmul(ps_ap, lhsT, rhs, start=True, stop=True,
                                 tile_position=(32 * j, 0))

            for h in range(2):
                # each matmul output starts at a PSUM bank boundary (the
                # hardware rejects sub-bank-offset matmul outputs)
                for jp in (0, 2):
                    ps = pM.tile([128, 2, 512], f32, tag="ps")
                    for jj in (0, 1):
                        mm(ps[:, jj, 0:C], jp + jj, h)
                    ENG[ev.pop(0)](so[:, h, jp:jp + 2, :], ps[:, :, 0:C])
                if split_h_dma:
                    nc.sync.dma_start(
                        outv[:, h:h + 1, :, col0:col0 + C],
                        so[:, h:h + 1, :, :])
            if not split_h_dma:
                nc.sync.dma_start(outv[:, :, :, col0:col0 + C], so[:])

        # ---------------- pipeline emission -------------------------------
        # PE warmup: keep the tensor engine busy from pos arrival so the
        # p-state ramp reaches full speed before the first real transpose
        def emit_warmup(n):
            # low-priority PE fillers: scheduler slots them into idle PE
            # slices, keeping the p-state ramp at full speed all run long
            _mark("warm")
            wfill = pW.tile([128, 128], f32)
            with tc.high_priority(offset=-10**6):
                for _ in range(n):
                    nc.tensor.transpose(wfill[0:32, 0:64], posA0[:, 0:4, :],
                                        posA0[:, 0:8, :])

        # target start times (us) keep the greedy scheduler from front-running
        # far-future work into engine streams ahead of urgent ops
        def at(us):
            return tc.tile_wait_until(us / 1000.0)

        emit_seeds(0)
        emit_warmup(110)
        emit_products(0)
        emit_tp(0)
        with at(3.6):
            emit_seeds(1)
        if not PSI_BF16:
            scopy(wtsr[:], wts[:])
        emit_mm(0, "adda", split_h_dma=True)
        with at(5.2):
            emit_products(1)
            emit_tp(1, "a")
        with at(5.3):
            emit_mm(1, "daad")
        with at(4.6):
            emit_seeds(2)          # (16,32)
        with at(6.6):
            emit_products(2)
            emit_tp(2, "d")
        with at(6.7):
            emit_mm(2, "adda")
        with at(6.8):
            emit_seeds(3)          # (32,64)
        with at(8.6):
            emit_products(3)
            emit_tp(3, "a")
        with at(8.7):
            emit_mm(3, "adaa")
        with at(8.8):
            emit_seeds(4)          # (64,108)
        PRT = {4: 11.4, 5: 14.4, 6: 17.4, 7: 20.4}
        for ci in range(4, len(CHUNKS)):
            with at(PRT[ci]):
                emit_products(ci)
                emit_tp(ci, "d" if ci % 2 else "a")
            with at(PRT[ci] + 0.1):
                emit_mm(ci, "adaa" if ci % 2 else "adda")


def _get_program():
    if "nc" not in _PROGRAM_CACHE:
        _PROGRAM_CACHE["nc"] = _build_program()
    return _PROGRAM_CACHE["nc"]


# ---------------------------------------------------------------- host wrapper
def _host_prep(position, coeffs):
    K = _fold_constants()
    Cs = (np.asarray(coeffs, np.float64).reshape(OUTC * INC, NB)
          * K[None, :]).astype(np.float32)
    W = np.zeros((128, 256), np.float32)
    for j in range(NGRP):
        W[32 * j:32 * j + NB, :] = Cs.T
    if PSI_BF16:
        import ml_dtypes
        W = W.astype(ml_dtypes.bfloat16)

    pts = np.asarray(position, np.float64).reshape(PTS, 3)
    r, th, ph = pts[:, 0], pts[:, 1], pts[:, 2]
    sp, cp = np.sin(ph), np.cos(ph)
    X = np.stack([np.sin(th), np.cos(th), sp, cp,
                  sp * cp, cp * cp - 0.5,
                  (cp * cp - 0.25) * sp, (cp * cp - 0.75) * cp,
                  r / 2.0, r / 3.0, r / 4.0, r], axis=1).astype(np.float32)
    pieces = []
    for k in range(NCORES):
        sl = X[k * CPTS:(k + 1) * CPTS]                # [13824, NPC]
        v = sl.reshape(NGRP, NROUND, 128, NPC)         # [j, c, p, col]
        v = np.transpose(v, (2, 3, 1, 0))              # [p, col, c, j]
        pos = v.reshape(128, NPC, F)                   # g = 4c + j
        pieces.append((np.ascontiguousarray(pos[:, :, 0:8]),
                       np.ascontiguousarray(pos[:, :, 8:16]),
                       np.ascontiguousarray(pos[:, :, 16:32]),
                       np.ascontiguousarray(pos[:, :, 32:108])))
    return pieces, W


def kernel(position, coeffs, _collect=None):
    from concourse.bass_utils import run_bass_kernel_spmd

    import ml_dtypes
    pieces, W = _host_prep(position, coeffs)
    ident = np.eye(128, dtype=ml_dtypes.bfloat16 if PSI_BF16 else np.float32)
    rootsT = np.tile(np.asarray(ROOTLIST, np.float32), (128, 1))
    in_maps = [{"posA0": pieces[k][0], "posA1": pieces[k][1],
                "posB1": pieces[k][2], "posB2": pieces[k][3],
                "wts": W, "ident": ident, "rootsT": rootsT}
               for k in range(NCORES)]
    nc = _get_program()
    try:
        res = run_bass_kernel_spmd(nc, in_maps, core_ids=list(range(NCORES)))
    except Exception:
        # transient NRT/axon failures usually clear on retry
        res = run_bass_kernel_spmd(nc, in_maps, core_ids=list(range(NCORES)))
    if _collect is not None:
        _collect.append(res)
    full = np.concatenate(
        [np.asarray(res.results[k]["out"]).astype(np.float32)
         .transpose(1, 0, 2, 3).reshape(256, CPTS)
         for k in range(NCORES)], axis=1)
    return full.reshape(OUTC, INC, OUTN, CONV_N)


# revision 71
# speedup vs baseline: 1.0819x; 1.0819x over previous
"""Trainium2 Bass kernel for nn_DCConv3dKernelPolynomials.

out[o,i,x,n] = sum_b basis_b(position[x,n]) * coeffs[o,i,b]

Strategy (per the sharding hint): shard the 110592 grid points across the 8
NeuronCores (13824 each), replicate the folded coefficient matrix.  The host
re-encodes each point as [sin t, cos t, sin p, cos p, r] (a coordinate
re-parametrization, like the host-folded normalization constants), so the
device needs no Sin activations: the single Exp-set table load hides under
the input DMAs, the pi/2-pi const-AP preamble disappears, and the prepacked
pos layout keeps DMA descriptors >= 512B.  Per core:
 - evaluate the 30 hydrogen-wavefunction basis functions point-major on
   DVE/ACT/GPSIMD (normalizations folded into the coefficients host-side,
   Laguerre polynomials factored into real linear roots),
 - PE-transpose psi into a (30 x points) layout, 4 point-groups packed into
   the 128 partitions,
 - row-tiled fp32r matmuls (K=30 per 32-row group) against the replicated
   coefficients -> PSUM, evacuate via DVE/ACT to SBUF, DMA out as bf16.
"""
import math

import numpy as np

OUTC, INC = 16, 16
OUTN, CONV_N = 4096, 27
NB = 30
NCORES = 8
PTS = OUTN * CONV_N            # 110592
CPTS = PTS // NCORES           # 13824 per core
NGRP = 4                       # point groups (matmul row tiling)
GPTS = CPTS // NGRP            # 3456 per group
NROUND = GPTS // 128           # 27 transpose rounds
NT = 7                         # output column chunks per group (6x512 + 384)
PI = math.pi


# ----------------------------------------------------------------- constants
def _qnums():
    lst = []
    for n in range(1, 5):
        for l in range(0, min(n, 4)):
            for m in range(-l, l + 1):
                if abs(m) <= 3:
                    lst.append((n, l, m))
    return lst


QNUMS = _qnums()


def _laguerre_coeffs(k, alpha):
    return [((-1.0) ** i) * math.comb(k + alpha, k - i) / math.factorial(i)
            for i in range(k + 1)]


def _radial_info(n, l):
    k = n - l - 1
    lag = _laguerre_coeffs(k, 2 * l + 1)
    cr = [lag[i] * (2.0 / n) ** i for i in range(k + 1)]
    norm_r = math.sqrt((2.0 / n) ** 3 * math.factorial(n - l - 1)
                       / (2.0 * n * math.factorial(n + l)))
    lead = cr[-1]
    K_rad = norm_r * (2.0 / n) ** l * lead
    roots = [] if k == 0 else sorted(float(x) for x in
                                     np.real(np.roots(np.array(cr[::-1]))))
    return roots, K_rad


_K_ANG = {(0, 0): 1.0, (1, 0): 1.0, (1, 1): -1.0,
          (2, 0): 1.5, (2, 1): -3.0, (2, 2): 3.0,
          (3, 0): 2.5, (3, 1): -7.5, (3, 2): 15.0, (3, 3): -15.0}
_TRIGFOLD = {0: 1.0, 1: 1.0, -1: 1.0, 2: 2.0, -2: 2.0, 3: 4.0, -3: 4.0}

ROOTS20 = _radial_info(2, 0)[0]
ROOTS30 = _radial_info(3, 0)[0]
ROOTS31 = _radial_info(3, 1)[0]
ROOTS40 = _radial_info(4, 0)[0]
ROOTS41 = _radial_info(4, 1)[0]
ROOTS42 = _radial_info(4, 2)[0]


def _fold_constants():
    K = np.zeros(NB)
    for b, (n, l, m) in enumerate(QNUMS):
        am = abs(m)
        _, K_rad = _radial_info(n, l)
        klm = math.sqrt((2.0 * l + 1.0) / (4.0 * PI)
                        * math.factorial(l - am) / math.factorial(l + am))
        K[b] = (K_rad * klm * (math.sqrt(2.0) if m != 0 else 1.0)
                * _K_ANG[(l, am)] * _TRIGFOLD[m])
    return K


# ------------------------------------------------------------- device program
_PROGRAM_CACHE = {}


def _build_program():
    import concourse.bacc as bacc
    import concourse.tile as tile
    from concourse import mybir

    f32 = mybir.dt.float32
    f32r = mybir.dt.float32r
    AF = mybir.ActivationFunctionType

    nc = bacc.Bacc("TRN2", debug=False, num_devices=NCORES)

    # host sends [sin t, cos t, sin p, cos p, r] prepacked contiguous:
    # no Sin table on device (single hidden Exp-set load), no pi/2,pi
    # const-AP preamble, and >=512B DMA descriptors
    pos_d = nc.dram_tensor("pos", [128, 5, NROUND * NGRP], f32,
                           kind="ExternalInput")
    wts_d = nc.dram_tensor("wts", [128, 256], f32, kind="ExternalInput")
    ident_d = nc.dram_tensor("ident", [128, 128], f32, kind="ExternalInput")
    out_d = nc.dram_tensor("out", [256, CPTS], mybir.dt.bfloat16,
                           kind="ExternalOutput")

    with tile.TileContext(nc) as tc:
        _kernel_body(tc, nc, out_d.ap(), pos_d.ap(), wts_d.ap(), ident_d.ap(),
                     f32, f32r, AF)
    nc.compile()
    return nc


def _kernel_body(tc, nc, out_ap, pos_ap, wts_ap, ident_ap, f32, f32r, AF):
    from contextlib import ExitStack
    from concourse import mybir
    Alu = mybir.AluOpType

    ctx = ExitStack()
    with ctx:
        const = ctx.enter_context(tc.tile_pool(name="const", bufs=1))
        feat = ctx.enter_context(tc.tile_pool(name="feat", bufs=1))
        pT = ctx.enter_context(tc.tile_pool(name="pT", bufs=2, space="PSUM"))
        pM = ctx.enter_context(tc.tile_pool(name="pM", bufs=3, space="PSUM"))
        stg = ctx.enter_context(tc.tile_pool(name="stg", bufs=8))

        bf16 = mybir.dt.bfloat16
        F = NROUND * NGRP       # 108 g-columns (g = c*4 + j)
        # pipeline segments: g-range -> t-chunks it covers (t needs g[16t:16t+16))
        SEGS = [(0, 16, [0]), (16, 48, [1, 2]), (48, F, [3, 4, 5, 6])]

        def ft(name):
            t = feat.tile([128, F], f32, tag=name)
            return t

        posT = feat.tile([128, 5, F], f32)
        nc.sync.dma_start(posT[:], pos_ap)
        sth = posT[:, 0, :]; u = posT[:, 1, :]
        s1 = posT[:, 2, :]; c1 = posT[:, 3, :]; r = posT[:, 4, :]
        wts = const.tile([128, 256], f32)
        nc.sync.dma_start(wts[:], wts_ap)
        ident = const.tile([128, 128], f32)
        nc.sync.dma_start(ident[:], ident_ap)
        wtsr = const.tile([128, 256], f32r)
        nc.scalar.copy(wtsr[:], wts[:])

        # psi point-major, one tile per segment: PMs[i][p, g-g0, bb]
        PMs = []
        for i, (g0, g1, _) in enumerate(SEGS):
            pm = feat.tile([128, g1 - g0, 32], f32, tag=f"PM{i}")
            nc.vector.memset(pm[:, :, NB:32], 0.0)
            PMs.append(pm)

        act = nc.scalar.activation
        stt = nc.vector.scalar_tensor_tensor
        tt = nc.vector.tensor_tensor
        ts = nc.vector.tensor_scalar
        gtt = nc.gpsimd.tensor_tensor

        # ---- seeds (trig from host; ACT only runs the Exp set) ----
        E2 = ft("E2"); E3 = ft("E3"); E4 = ft("E4")
        act(E4[:], r[:], AF.Exp, scale=-0.25)
        act(E3[:], r[:], AF.Exp, scale=-1.0 / 3.0)
        act(E2[:], r[:], AF.Exp, scale=-0.5)
        u2 = ft("u2"); stsq = ft("stsq"); c1sq = ft("c1sq")
        tt(c1sq[:], c1[:], c1[:], Alu.mult)
        tt(u2[:], u[:], u[:], Alu.mult)
        tt(stsq[:], sth[:], sth[:], Alu.mult)

        # ---- radial (TT sub-chain on GPSIMD, stt stays on DVE) ----
        E2r = ft("E2r"); E3r = ft("E3r"); E4r = ft("E4r")
        gtt(E2r[:], E2[:], r[:], Alu.mult)
        gtt(E3r[:], E3[:], r[:], Alu.mult)
        gtt(E4r[:], E4[:], r[:], Alu.mult)
        R31 = ft("R31")
        stt(R31[:], r[:], ROOTS31[0], E3r[:], Alu.subtract, Alu.mult)
        R32 = ft("R32")
        gtt(R32[:], E3r[:], r[:], Alu.mult)
        E4r2 = ft("E4r2")
        gtt(E4r2[:], E4r[:], r[:], Alu.mult)
        R41a = ft("R41a"); R41 = ft("R41")
        stt(R41a[:], r[:], ROOTS41[0], E4r[:], Alu.subtract, Alu.mult)
        stt(R41[:], r[:], ROOTS41[1], R41a[:], Alu.subtract, Alu.mult)
        R42 = ft("R42")
        stt(R42[:], r[:], ROOTS42[0], E4r2[:], Alu.subtract, Alu.mult)
        R43 = ft("R43")
        gtt(R43[:], E4r2[:], r[:], Alu.mult)
        t35 = ft("t35")
        stt(t35[:], r[:], ROOTS30[0], E3[:], Alu.subtract, Alu.mult)
        t41 = ft("t41"); t42 = ft("t42")
        stt(t41[:], r[:], ROOTS40[0], E4[:], Alu.subtract, Alu.mult)
        stt(t42[:], r[:], ROOTS40[1], t41[:], Alu.subtract, Alu.mult)

        # ---- trig ladders / angular (late-consumer ops on GPSIMD) ----
        c2t = ft("c2t")
        ts(c2t[:], c1sq[:], -0.5, None, Alu.add)            # cos(2phi)/2
        s2t = ft("s2t")
        tt(s2t[:], s1[:], c1[:], Alu.mult)                  # sin(2phi)/2
        c3t = ft("c3t")
        stt(c3t[:], c1sq[:], 0.75, c1[:], Alu.subtract, Alu.mult)   # cos3/4
        s3t = ft("s3t")
        stt(s3t[:], c1sq[:], 0.25, s1[:], Alu.subtract, Alu.mult)   # sin3/4
        p20 = ft("p20")
        ts(p20[:], u2[:], -1.0 / 3.0, None, Alu.add)
        p30 = ft("p30")
        stt(p30[:], u2[:], 0.6, u[:], Alu.subtract, Alu.mult)
        p33 = ft("p33")
        gtt(p33[:], sth[:], stsq[:], Alu.mult)              # sin^3(theta)
        A1c = ft("A1c"); A1s = ft("A1s")
        tt(A1c[:], sth[:], c1[:], Alu.mult)
        tt(A1s[:], sth[:], s1[:], Alu.mult)
        A2c1 = ft("A2c1"); A2s1 = ft("A2s1")
        tt(A2c1[:], u[:], A1c[:], Alu.mult)
        tt(A2s1[:], u[:], A1s[:], Alu.mult)
        A2c2 = ft("A2c2"); A2s2 = ft("A2s2")
        tt(A2c2[:], stsq[:], c2t[:], Alu.mult)
        tt(A2s2[:], stsq[:], s2t[:], Alu.mult)
        A3c1 = ft("A3c1"); A3s1 = ft("A3s1")
        stt(A3c1[:], u2[:], 0.2, A1c[:], Alu.subtract, Alu.mult)
        stt(A3s1[:], u2[:], 0.2, A1s[:], Alu.subtract, Alu.mult)
        A3c2 = ft("A3c2"); A3s2 = ft("A3s2")
        gtt(A3c2[:], u[:], A2c2[:], Alu.mult)
        gtt(A3s2[:], u[:], A2s2[:], Alu.mult)
        A3c3 = ft("A3c3"); A3s3 = ft("A3s3")
        gtt(A3c3[:], p33[:], c3t[:], Alu.mult)
        gtt(A3s3[:], p33[:], s3t[:], Alu.mult)

        # ---- pipelined: psi seg-products, transposes, matmuls, DMA ----
        psiT = feat.tile([128, GPTS], f32r)
        out3 = out_ap.rearrange("o (j p) -> o j p", j=NGRP)

        def psi_seg(si):
            g0, g1, _ = SEGS[si]
            PM = PMs[si]
            sl = slice(g0, g1)

            def pslot(b):
                return PM[:, :, b]

            act(pslot(0), r[:, sl], AF.Exp, scale=-1.0)
            stt(pslot(1), r[:, sl], ROOTS20[0], E2[:, sl],
                Alu.subtract, Alu.mult)
            tt(pslot(2), E2r[:, sl], A1s[:, sl], Alu.mult)
            tt(pslot(3), E2r[:, sl], u[:, sl], Alu.mult)
            tt(pslot(4), E2r[:, sl], A1c[:, sl], Alu.mult)
            stt(pslot(5), r[:, sl], ROOTS30[1], t35[:, sl],
                Alu.subtract, Alu.mult)
            tt(pslot(6), R31[:, sl], A1s[:, sl], Alu.mult)
            tt(pslot(7), R31[:, sl], u[:, sl], Alu.mult)
            tt(pslot(8), R31[:, sl], A1c[:, sl], Alu.mult)
            gtt(pslot(9), R32[:, sl], A2s2[:, sl], Alu.mult)
            tt(pslot(10), R32[:, sl], A2s1[:, sl], Alu.mult)
            tt(pslot(11), R32[:, sl], p20[:, sl], Alu.mult)
            tt(pslot(12), R32[:, sl], A2c1[:, sl], Alu.mult)
            gtt(pslot(13), R32[:, sl], A2c2[:, sl], Alu.mult)
            stt(pslot(14), r[:, sl], ROOTS40[2], t42[:, sl],
                Alu.subtract, Alu.mult)
            tt(pslot(15), R41[:, sl], A1s[:, sl], Alu.mult)
            tt(pslot(16), R41[:, sl], u[:, sl], Alu.mult)
            tt(pslot(17), R41[:, sl], A1c[:, sl], Alu.mult)
            gtt(pslot(18), R42[:, sl], A2s2[:, sl], Alu.mult)
            tt(pslot(19), R42[:, sl], A2s1[:, sl], Alu.mult)
            tt(pslot(20), R42[:, sl], p20[:, sl], Alu.mult)
            tt(pslot(21), R42[:, sl], A2c1[:, sl], Alu.mult)
            gtt(pslot(22), R42[:, sl], A2c2[:, sl], Alu.mult)
            gtt(pslot(23), R43[:, sl], A3s3[:, sl], Alu.mult)
            gtt(pslot(24), R43[:, sl], A3s2[:, sl], Alu.mult)
            tt(pslot(25), R43[:, sl], A3s1[:, sl], Alu.mult)
            tt(pslot(26), R43[:, sl], p30[:, sl], Alu.mult)
            tt(pslot(27), R43[:, sl], A3c1[:, sl], Alu.mult)
            gtt(pslot(28), R43[:, sl], A3c2[:, sl], Alu.mult)
            gtt(pslot(29), R43[:, sl], A3c3[:, sl], Alu.mult)

        def transposes_seg(si):
            g0, g1, _ = SEGS[si]
            PM = PMs[si]
            c_lo, c_hi = g0 // 4, g1 // 4
            for cb in range(c_lo, c_hi, 4):
                nb4 = min(4, c_hi - cb)
                tp = pT.tile([128, 512], f32, tag="tp")
                for ci in range(nb4):
                    c = cb + ci
                    nc.tensor.transpose(
                        tp[:, ci * 128:(ci + 1) * 128],
                        PM[:, 4 * c - g0:4 * c - g0 + 4, :], ident[:])
                dst = psiT[:, cb * 128:(cb + nb4) * 128]
                if (cb // 4) % 2 == 0:
                    nc.scalar.copy(dst, tp[:, :nb4 * 128])
                else:
                    nc.vector.tensor_copy(dst, tp[:, :nb4 * 128])

        evac_state = [0]

        def mm_chunk(t, dve_evac_mod):
            n = min(512, GPTS - t * 512)
            for h in range(2):
                so = stg.tile([128, 2048], bf16, tag="so")
                so4 = so.rearrange("p (j q) -> p j q", q=512)
                for jp in (0, 2):
                    ps = pM.tile([128, 1024], f32, tag="ps")
                    for jj in (0, 1):
                        j = jp + jj
                        lhsT = wtsr[32 * j:32 * j + NB,
                                    128 * h:128 * (h + 1)]
                        rhs = psiT[32 * j:32 * j + NB, t * 512:t * 512 + n]
                        nc.tensor.matmul(ps[:, jj * 512:jj * 512 + n],
                                         lhsT, rhs, start=True, stop=True,
                                         tile_position=(32 * j, 0))
                    # one wide copy covers both j outputs; the [n:512] gap
                    # is never DMA'd so copying it is harmless
                    w = 512 + n
                    i = evac_state[0]
                    evac_state[0] += 1
                    if i % 3 == 0:
                        nc.vector.tensor_copy(
                            so[:, jp * 512:jp * 512 + w], ps[:, :w])
                    else:
                        nc.scalar.copy(
                            so[:, jp * 512:jp * 512 + w], ps[:, :w])
                dst = out3[128 * h:128 * (h + 1), :, t * 512:t * 512 + n]
                nc.sync.dma_start(dst, so4[:, :, :n])

        # emission order = scheduler priority: psi products of the next
        # segment outrank evacuations of the previous one on DVE
        psi_seg(0)
        transposes_seg(0)
        psi_seg(1)
        mm_chunk(0, 0)          # early evacs: ACT only
        transposes_seg(1)
        psi_seg(2)
        mm_chunk(1, 0)
        mm_chunk(2, 0)
        transposes_seg(2)
        for t in (3, 4, 5, 6):
            mm_chunk(t, 2)      # late evacs: alternate DVE/ACT


def _get_program():
    if "nc" not in _PROGRAM_CACHE:
        _PROGRAM_CACHE["nc"] = _build_program()
    return _PROGRAM_CACHE["nc"]


# ---------------------------------------------------------------- host wrapper
def _host_prep(position, coeffs):
    K = _fold_constants()
    Cs = (np.asarray(coeffs, np.float64).reshape(OUTC * INC, NB)
          * K[None, :]).astype(np.float32)
    W = np.zeros((128, 256), np.float32)
    for j in range(NGRP):
        W[32 * j:32 * j + NB, :] = Cs.T
    pts = np.asarray(position, np.float64).reshape(PTS, 3)
    r, th, ph = pts[:, 0], pts[:, 1], pts[:, 2]
    X = np.stack([np.sin(th), np.cos(th), np.sin(ph), np.cos(ph), r],
                 axis=1).astype(np.float32)
    pos_cores = []
    for k in range(NCORES):
        sl = X[k * CPTS:(k + 1) * CPTS]
        v = sl.reshape(NGRP, NROUND, 128, 5)       # [j, c, p, coord]
        v = np.transpose(v, (2, 3, 1, 0))          # [p, coord, c, j]
        pos_cores.append(np.ascontiguousarray(
            v.reshape(128, 5, NROUND * NGRP)))
    return pos_cores, W


def kernel(position, coeffs, _collect=None):
    from concourse.bass_utils import run_bass_kernel_spmd

    pos_cores, W = _host_prep(position, coeffs)
    ident = np.eye(128, dtype=np.float32)
    in_maps = [{"pos": pos_cores[k], "wts": W, "ident": ident}
               for k in range(NCORES)]
    nc = _get_program()
    try:
        res = run_bass_kernel_spmd(nc, in_maps, core_ids=list(range(NCORES)))
    except Exception:
        # transient NRT/axon failures (e.g. a wedged core from a previous
        # process) usually clear on retry
        res = run_bass_kernel_spmd(nc, in_maps, core_ids=list(range(NCORES)))
    if _collect is not None:
        _collect.append(res)
    full = np.concatenate(
        [np.asarray(res.results[k]["out"]).astype(np.float32)
         for k in range(NCORES)], axis=1)
    return full.reshape(OUTC, INC, OUTN, CONV_N)



# revision 74
# speedup vs baseline: 1.1055x; 1.0219x over previous
"""Trainium2 Bass kernel for nn_DCConv3dKernelPolynomials.

out[o,i,x,n] = sum_b basis_b(position[x,n]) * coeffs[o,i,b]

Strategy (per the sharding hint): shard the 110592 grid points across the 8
NeuronCores (13824 each), replicate the folded coefficient matrix.  The host
re-encodes each point as [sin t, cos t, sin p, cos p, r] (a coordinate
re-parametrization, like the host-folded normalization constants), so the
device needs no Sin activations: the single Exp-set table load hides under
the input DMAs, the pi/2-pi const-AP preamble disappears, and the prepacked
pos layout keeps DMA descriptors >= 512B.  Per core:
 - evaluate the 30 hydrogen-wavefunction basis functions point-major on
   DVE/ACT/GPSIMD (normalizations folded into the coefficients host-side,
   Laguerre polynomials factored into real linear roots),
 - PE-transpose psi into a (30 x points) layout, 4 point-groups packed into
   the 128 partitions,
 - row-tiled fp32r matmuls (K=30 per 32-row group) against the replicated
   coefficients -> PSUM, evacuate via DVE/ACT to SBUF, DMA out as bf16.
"""
import math

import numpy as np

OUTC, INC = 16, 16
OUTN, CONV_N = 4096, 27
NB = 30
NCORES = 8
PTS = OUTN * CONV_N            # 110592
CPTS = PTS // NCORES           # 13824 per core
NGRP = 4                       # point groups (matmul row tiling)
GPTS = CPTS // NGRP            # 3456 per group
NROUND = GPTS // 128           # 27 transpose rounds
NT = 7                         # output column chunks per group (6x512 + 384)
PI = math.pi


# ----------------------------------------------------------------- constants
def _qnums():
    lst = []
    for n in range(1, 5):
        for l in range(0, min(n, 4)):
            for m in range(-l, l + 1):
                if abs(m) <= 3:
                    lst.append((n, l, m))
    return lst


QNUMS = _qnums()


def _laguerre_coeffs(k, alpha):
    return [((-1.0) ** i) * math.comb(k + alpha, k - i) / math.factorial(i)
            for i in range(k + 1)]


def _radial_info(n, l):
    k = n - l - 1
    lag = _laguerre_coeffs(k, 2 * l + 1)
    cr = [lag[i] * (2.0 / n) ** i for i in range(k + 1)]
    norm_r = math.sqrt((2.0 / n) ** 3 * math.factorial(n - l - 1)
                       / (2.0 * n * math.factorial(n + l)))
    lead = cr[-1]
    K_rad = norm_r * (2.0 / n) ** l * lead
    roots = [] if k == 0 else sorted(float(x) for x in
                                     np.real(np.roots(np.array(cr[::-1]))))
    return roots, K_rad


_K_ANG = {(0, 0): 1.0, (1, 0): 1.0, (1, 1): -1.0,
          (2, 0): 1.5, (2, 1): -3.0, (2, 2): 3.0,
          (3, 0): 2.5, (3, 1): -7.5, (3, 2): 15.0, (3, 3): -15.0}
_TRIGFOLD = {0: 1.0, 1: 1.0, -1: 1.0, 2: 2.0, -2: 2.0, 3: 4.0, -3: 4.0}

ROOTS20 = _radial_info(2, 0)[0]
ROOTS30 = _radial_info(3, 0)[0]
ROOTS31 = _radial_info(3, 1)[0]
ROOTS40 = _radial_info(4, 0)[0]
ROOTS41 = _radial_info(4, 1)[0]
ROOTS42 = _radial_info(4, 2)[0]


def _fold_constants():
    K = np.zeros(NB)
    for b, (n, l, m) in enumerate(QNUMS):
        am = abs(m)
        _, K_rad = _radial_info(n, l)
        klm = math.sqrt((2.0 * l + 1.0) / (4.0 * PI)
                        * math.factorial(l - am) / math.factorial(l + am))
        K[b] = (K_rad * klm * (math.sqrt(2.0) if m != 0 else 1.0)
                * _K_ANG[(l, am)] * _TRIGFOLD[m])
    return K


# ------------------------------------------------------------- device program
_PROGRAM_CACHE = {}


def _build_program():
    import concourse.bacc as bacc
    import concourse.tile as tile
    from concourse import mybir

    f32 = mybir.dt.float32
    f32r = mybir.dt.float32r
    AF = mybir.ActivationFunctionType

    nc = bacc.Bacc("TRN2", debug=False, num_devices=NCORES)

    # host sends [sin t, cos t, sin p, cos p, r] prepacked contiguous:
    # no Sin table on device (single hidden Exp-set load), no pi/2,pi
    # const-AP preamble, and >=512B DMA descriptors
    pos_d = nc.dram_tensor("pos", [128, 5, NROUND * NGRP], f32,
                           kind="ExternalInput")
    wts_d = nc.dram_tensor("wts", [128, 256], f32, kind="ExternalInput")
    ident_d = nc.dram_tensor("ident", [128, 128], f32, kind="ExternalInput")
    out_d = nc.dram_tensor("out", [256, CPTS], mybir.dt.bfloat16,
                           kind="ExternalOutput")

    with tile.TileContext(nc) as tc:
        _kernel_body(tc, nc, out_d.ap(), pos_d.ap(), wts_d.ap(), ident_d.ap(),
                     f32, f32r, AF)
    nc.compile()
    return nc


def _kernel_body(tc, nc, out_ap, pos_ap, wts_ap, ident_ap, f32, f32r, AF):
    from contextlib import ExitStack
    from concourse import mybir
    Alu = mybir.AluOpType

    ctx = ExitStack()
    with ctx:
        const = ctx.enter_context(tc.tile_pool(name="const", bufs=1))
        feat = ctx.enter_context(tc.tile_pool(name="feat", bufs=1))
        pT = ctx.enter_context(tc.tile_pool(name="pT", bufs=2, space="PSUM"))
        pM = ctx.enter_context(tc.tile_pool(name="pM", bufs=3, space="PSUM"))
        stg = ctx.enter_context(tc.tile_pool(name="stg", bufs=8))

        bf16 = mybir.dt.bfloat16
        F = NROUND * NGRP       # 108 g-columns (g = c*4 + j)
        # pipeline segments: g-range -> t-chunks it covers (t needs g[16t:16t+16))
        SEGS = [(0, 16, [0]), (16, 48, [1, 2]), (48, F, [3, 4, 5, 6])]

        def ft(name):
            t = feat.tile([128, F], f32, tag=name)
            return t

        posT = feat.tile([128, 5, F], f32)
        nc.sync.dma_start(posT[:], pos_ap)
        sth = posT[:, 0, :]; u = posT[:, 1, :]
        s1 = posT[:, 2, :]; c1 = posT[:, 3, :]; r = posT[:, 4, :]
        wts = const.tile([128, 256], f32)
        nc.sync.dma_start(wts[:], wts_ap)
        ident = const.tile([128, 128], f32)
        nc.sync.dma_start(ident[:], ident_ap)
        wtsr = const.tile([128, 256], f32r)
        nc.scalar.copy(wtsr[:], wts[:])

        # psi point-major, one tile per segment: PMs[i][p, g-g0, bb]
        PMs = []
        for i, (g0, g1, _) in enumerate(SEGS):
            pm = feat.tile([128, g1 - g0, 32], f32, tag=f"PM{i}")
            nc.vector.memset(pm[:, :, NB:32], 0.0)
            PMs.append(pm)

        act = nc.scalar.activation
        stt = nc.vector.scalar_tensor_tensor
        tt = nc.vector.tensor_tensor
        ts = nc.vector.tensor_scalar
        gtt = nc.gpsimd.tensor_tensor

        # ---- prep chain, emitted in two column ranges so segment 0's
        # critical path (16 g-cols) clears ~4x sooner than the full width ----
        E2 = ft("E2"); E3 = ft("E3"); E4 = ft("E4")
        u2 = ft("u2"); stsq = ft("stsq"); c1sq = ft("c1sq")
        E2r = ft("E2r"); E3r = ft("E3r"); E4r = ft("E4r")
        R31 = ft("R31"); R32 = ft("R32"); E4r2 = ft("E4r2")
        R41a = ft("R41a"); R41 = ft("R41"); R42 = ft("R42"); R43 = ft("R43")
        t35 = ft("t35"); t41 = ft("t41"); t42 = ft("t42")
        c2t = ft("c2t"); s2t = ft("s2t"); c3t = ft("c3t"); s3t = ft("s3t")
        p20 = ft("p20"); p30 = ft("p30"); p33 = ft("p33")
        A1c = ft("A1c"); A1s = ft("A1s")
        A2c1 = ft("A2c1"); A2s1 = ft("A2s1")
        A2c2 = ft("A2c2"); A2s2 = ft("A2s2")
        A3c1 = ft("A3c1"); A3s1 = ft("A3s1")
        A3c2 = ft("A3c2"); A3s2 = ft("A3s2")
        A3c3 = ft("A3c3"); A3s3 = ft("A3s3")

        def emit_prep(a, b):
            q = slice(a, b)
            rq = posT[:, 4, q]
            sthq = posT[:, 0, q]; uq = posT[:, 1, q]
            s1q = posT[:, 2, q]; c1q = posT[:, 3, q]
            # seeds (trig from host; ACT only runs the Exp set)
            act(E4[:, q], rq, AF.Exp, scale=-0.25)
            act(E3[:, q], rq, AF.Exp, scale=-1.0 / 3.0)
            act(E2[:, q], rq, AF.Exp, scale=-0.5)
            tt(c1sq[:, q], c1q, c1q, Alu.mult)
            tt(u2[:, q], uq, uq, Alu.mult)
            tt(stsq[:, q], sthq, sthq, Alu.mult)
            # radial (TT sub-chain on GPSIMD, stt stays on DVE)
            gtt(E2r[:, q], E2[:, q], rq, Alu.mult)
            gtt(E3r[:, q], E3[:, q], rq, Alu.mult)
            gtt(E4r[:, q], E4[:, q], rq, Alu.mult)
            stt(R31[:, q], rq, ROOTS31[0], E3r[:, q], Alu.subtract, Alu.mult)
            gtt(R32[:, q], E3r[:, q], rq, Alu.mult)
            gtt(E4r2[:, q], E4r[:, q], rq, Alu.mult)
            stt(R41a[:, q], rq, ROOTS41[0], E4r[:, q], Alu.subtract, Alu.mult)
            stt(R41[:, q], rq, ROOTS41[1], R41a[:, q], Alu.subtract, Alu.mult)
            stt(R42[:, q], rq, ROOTS42[0], E4r2[:, q], Alu.subtract, Alu.mult)
            gtt(R43[:, q], E4r2[:, q], rq, Alu.mult)
            stt(t35[:, q], rq, ROOTS30[0], E3[:, q], Alu.subtract, Alu.mult)
            stt(t41[:, q], rq, ROOTS40[0], E4[:, q], Alu.subtract, Alu.mult)
            stt(t42[:, q], rq, ROOTS40[1], t41[:, q], Alu.subtract, Alu.mult)
            # trig ladders / angular (late-consumer ops on GPSIMD)
            ts(c2t[:, q], c1sq[:, q], -0.5, None, Alu.add)
            tt(s2t[:, q], s1q, c1q, Alu.mult)
            stt(c3t[:, q], c1sq[:, q], 0.75, c1q, Alu.subtract, Alu.mult)
            stt(s3t[:, q], c1sq[:, q], 0.25, s1q, Alu.subtract, Alu.mult)
            ts(p20[:, q], u2[:, q], -1.0 / 3.0, None, Alu.add)
            stt(p30[:, q], u2[:, q], 0.6, uq, Alu.subtract, Alu.mult)
            gtt(p33[:, q], sthq, stsq[:, q], Alu.mult)
            tt(A1c[:, q], sthq, c1q, Alu.mult)
            tt(A1s[:, q], sthq, s1q, Alu.mult)
            tt(A2c1[:, q], uq, A1c[:, q], Alu.mult)
            tt(A2s1[:, q], uq, A1s[:, q], Alu.mult)
            tt(A2c2[:, q], stsq[:, q], c2t[:, q], Alu.mult)
            tt(A2s2[:, q], stsq[:, q], s2t[:, q], Alu.mult)
            stt(A3c1[:, q], u2[:, q], 0.2, A1c[:, q], Alu.subtract, Alu.mult)
            stt(A3s1[:, q], u2[:, q], 0.2, A1s[:, q], Alu.subtract, Alu.mult)
            gtt(A3c2[:, q], uq, A2c2[:, q], Alu.mult)
            gtt(A3s2[:, q], uq, A2s2[:, q], Alu.mult)
            gtt(A3c3[:, q], p33[:, q], c3t[:, q], Alu.mult)
            gtt(A3s3[:, q], p33[:, q], s3t[:, q], Alu.mult)

        # ---- pipelined: psi seg-products, transposes, matmuls, DMA ----
        psiT = feat.tile([128, GPTS], f32r)
        out3 = out_ap.rearrange("o (j p) -> o j p", j=NGRP)

        def psi_seg(si):
            g0, g1, _ = SEGS[si]
            PM = PMs[si]
            sl = slice(g0, g1)

            def pslot(b):
                return PM[:, :, b]

            act(pslot(0), r[:, sl], AF.Exp, scale=-1.0)
            stt(pslot(1), r[:, sl], ROOTS20[0], E2[:, sl],
                Alu.subtract, Alu.mult)
            tt(pslot(2), E2r[:, sl], A1s[:, sl], Alu.mult)
            tt(pslot(3), E2r[:, sl], u[:, sl], Alu.mult)
            tt(pslot(4), E2r[:, sl], A1c[:, sl], Alu.mult)
            stt(pslot(5), r[:, sl], ROOTS30[1], t35[:, sl],
                Alu.subtract, Alu.mult)
            tt(pslot(6), R31[:, sl], A1s[:, sl], Alu.mult)
            tt(pslot(7), R31[:, sl], u[:, sl], Alu.mult)
            tt(pslot(8), R31[:, sl], A1c[:, sl], Alu.mult)
            gtt(pslot(9), R32[:, sl], A2s2[:, sl], Alu.mult)
            tt(pslot(10), R32[:, sl], A2s1[:, sl], Alu.mult)
            tt(pslot(11), R32[:, sl], p20[:, sl], Alu.mult)
            tt(pslot(12), R32[:, sl], A2c1[:, sl], Alu.mult)
            gtt(pslot(13), R32[:, sl], A2c2[:, sl], Alu.mult)
            stt(pslot(14), r[:, sl], ROOTS40[2], t42[:, sl],
                Alu.subtract, Alu.mult)
            tt(pslot(15), R41[:, sl], A1s[:, sl], Alu.mult)
            tt(pslot(16), R41[:, sl], u[:, sl], Alu.mult)
            tt(pslot(17), R41[:, sl], A1c[:, sl], Alu.mult)
            gtt(pslot(18), R42[:, sl], A2s2[:, sl], Alu.mult)
            tt(pslot(19), R42[:, sl], A2s1[:, sl], Alu.mult)
            tt(pslot(20), R42[:, sl], p20[:, sl], Alu.mult)
            tt(pslot(21), R42[:, sl], A2c1[:, sl], Alu.mult)
            gtt(pslot(22), R42[:, sl], A2c2[:, sl], Alu.mult)
            gtt(pslot(23), R43[:, sl], A3s3[:, sl], Alu.mult)
            gtt(pslot(24), R43[:, sl], A3s2[:, sl], Alu.mult)
            tt(pslot(25), R43[:, sl], A3s1[:, sl], Alu.mult)
            tt(pslot(26), R43[:, sl], p30[:, sl], Alu.mult)
            tt(pslot(27), R43[:, sl], A3c1[:, sl], Alu.mult)
            gtt(pslot(28), R43[:, sl], A3c2[:, sl], Alu.mult)
            gtt(pslot(29), R43[:, sl], A3c3[:, sl], Alu.mult)

        def transposes_seg(si):
            g0, g1, _ = SEGS[si]
            PM = PMs[si]
            c_lo, c_hi = g0 // 4, g1 // 4
            for cb in range(c_lo, c_hi, 4):
                nb4 = min(4, c_hi - cb)
                tp = pT.tile([128, 512], f32, tag="tp")
                for ci in range(nb4):
                    c = cb + ci
                    nc.tensor.transpose(
                        tp[:, ci * 128:(ci + 1) * 128],
                        PM[:, 4 * c - g0:4 * c - g0 + 4, :], ident[:])
                dst = psiT[:, cb * 128:(cb + nb4) * 128]
                if (cb // 4) % 2 == 0:
                    nc.scalar.copy(dst, tp[:, :nb4 * 128])
                else:
                    nc.vector.tensor_copy(dst, tp[:, :nb4 * 128])

        evac_state = [0]

        def mm_chunk(t, dve_evac_mod):
            n = min(512, GPTS - t * 512)
            for h in range(2):
                so = stg.tile([128, 2048], bf16, tag="so")
                so4 = so.rearrange("p (j q) -> p j q", q=512)
                for jp in (0, 2):
                    ps = pM.tile([128, 1024], f32, tag="ps")
                    for jj in (0, 1):
                        j = jp + jj
                        lhsT = wtsr[32 * j:32 * j + NB,
                                    128 * h:128 * (h + 1)]
                        rhs = psiT[32 * j:32 * j + NB, t * 512:t * 512 + n]
                        nc.tensor.matmul(ps[:, jj * 512:jj * 512 + n],
                                         lhsT, rhs, start=True, stop=True,
                                         tile_position=(32 * j, 0))
                    # one wide copy covers both j outputs; the [n:512] gap
                    # is never DMA'd so copying it is harmless
                    w = 512 + n
                    i = evac_state[0]
                    evac_state[0] += 1
                    if i % 3 == 0:
                        nc.vector.tensor_copy(
                            so[:, jp * 512:jp * 512 + w], ps[:, :w])
                    else:
                        nc.scalar.copy(
                            so[:, jp * 512:jp * 512 + w], ps[:, :w])
                dst = out3[128 * h:128 * (h + 1), :, t * 512:t * 512 + n]
                nc.sync.dma_start(dst, so4[:, :, :n])

        # emission order = scheduler priority: psi products of the next
        # segment outrank evacuations of the previous one on DVE
        emit_prep(0, 16)
        psi_seg(0)
        transposes_seg(0)
        emit_prep(16, F)
        psi_seg(1)
        mm_chunk(0, 0)          # early evacs: ACT only
        transposes_seg(1)
        psi_seg(2)
        mm_chunk(1, 0)
        mm_chunk(2, 0)
        transposes_seg(2)
        for t in (3, 4, 5, 6):
            mm_chunk(t, 2)      # late evacs: alternate DVE/ACT


def _get_program():
    if "nc" not in _PROGRAM_CACHE:
        _PROGRAM_CACHE["nc"] = _build_program()
    return _PROGRAM_CACHE["nc"]


# ---------------------------------------------------------------- host wrapper
def _host_prep(position, coeffs):
    K = _fold_constants()
    Cs = (np.asarray(coeffs, np.float64).reshape(OUTC * INC, NB)
          * K[None, :]).astype(np.float32)
    W = np.zeros((128, 256), np.float32)
    for j in range(NGRP):
        W[32 * j:32 * j + NB, :] = Cs.T
    pts = np.asarray(position, np.float64).reshape(PTS, 3)
    r, th, ph = pts[:, 0], pts[:, 1], pts[:, 2]
    X = np.stack([np.sin(th), np.cos(th), np.sin(ph), np.cos(ph), r],
                 axis=1).astype(np.float32)
    pos_cores = []
    for k in range(NCORES):
        sl = X[k * CPTS:(k + 1) * CPTS]
        v = sl.reshape(NGRP, NROUND, 128, 5)       # [j, c, p, coord]
        v = np.transpose(v, (2, 3, 1, 0))          # [p, coord, c, j]
        pos_cores.append(np.ascontiguousarray(
            v.reshape(128, 5, NROUND * NGRP)))
    return pos_cores, W


def kernel(position, coeffs, _collect=None):
    from concourse.bass_utils import run_bass_kernel_spmd

    pos_cores, W = _host_prep(position, coeffs)
    ident = np.eye(128, dtype=np.float32)
    in_maps = [{"pos": pos_cores[k], "wts": W, "ident": ident}
               for k in range(NCORES)]
    nc = _get_program()
    try:
        res = run_bass_kernel_spmd(nc, in_maps, core_ids=list(range(NCORES)))
    except Exception:
        # transient NRT/axon failures (e.g. a wedged core from a previous
        # process) usually clear on retry
        res = run_bass_kernel_spmd(nc, in_maps, core_ids=list(range(NCORES)))
    if _collect is not None:
        _collect.append(res)
    full = np.concatenate(
        [np.asarray(res.results[k]["out"]).astype(np.float32)
         for k in range(NCORES)], axis=1)
    return full.reshape(OUTC, INC, OUTN, CONV_N)

